# revision 1
# baseline (speedup 1.0000x reference)
"""Trainium2 Bass kernel for nn_MoEElementFusion (moe_routing).

Strategy (8 NeuronCores, SPMD, two launches with host routing in between):
  Phase 1 (token-data-parallel): each core takes 1/8 of the 8192 (view,token)
  columns and computes in fp16 on the PE (psum fp32):
      h  = x @ proj_w + proj_b            (feature-major, weights stationary)
      r  = h @ router_w
      d2 = |r|^2 - 2 r.keys^T + |keys|^2  (rk/rr/kk assembled on the PE; the
                                           |k|^2 / rr K=1-matmuls stay fp32)
  Outputs h^T (fp16) and d2^T (fp32).

  Host: tokens whose 4th/5th logit gap is under REPAIR_MARGIN get their d2
  row recomputed exactly in fp32 (so the top-4 SET matches the fp32
  reference bit-for-bit); then logits = -sqrt(max(d2,0)), stable top-4 and
  softmax gates in fp32. A balanced slot plan cuts each expert's selected
  tokens into slots of L=512 columns; the 8 cores each run S identical
  slots (expert weights + gathered h columns + gate rows are per-core
  input data, so arbitrarily skewed routing stays perfectly balanced).

  Phase 2 (compiled at runtime once S is known): per slot, FFN in fp16
  (1 cycle/row on the PE):
      out^T = (w2^T-mm(gelu(w1^T-mm(h_gathered^T) + b1)) + b2) * gates
  feature-major throughout; gates applied from a pre-broadcast [128, C]
  row via one fused DVE scalar_tensor_tensor; weights double/triple
  buffered and streamed per-slot over split HWDGE queues.

  Host combine: fused[:, tok] += out columns per slot; sum the two views.
"""

import math
import os

import numpy as np

import concourse.bass as bass
import concourse.bacc as bacc
import concourse.mybir as mybir
import concourse.tile as tile
from concourse.bass_utils import run_bass_kernel_spmd

# Problem dims (hardcoded per spec)
V, B, T, D, E, K = 2, 4, 1024, 512, 16, 4
H = 4 * D
N = B * T          # tokens per view
NT = V * N         # total (view, token) columns = 8192
NC = 8             # cores
PC = NT // NC      # phase-1 columns per core = 1024
L = 512            # phase-2 slot length (columns)

F32 = mybir.dt.float32
F32R = mybir.dt.float32r
AF = mybir.ActivationFunctionType
ALU = mybir.AluOpType

DK = D // 128      # 4 k-tiles over D
HK = H // 128      # 16 k-tiles over H

# Phase-2 FFN in fp16 (1 cycle/row on the PE vs 2 for fp32; psum stays fp32).
P2_F16 = os.environ.get("KP2F32") != "1"
# Phase-1 proj/router in fp16; borderline top-4 selections (logit gap below
# REPAIR_MARGIN) are recomputed exactly on host in fp32.
P1_F16 = os.environ.get("KP1F32") != "1"
REPAIR_MARGIN = 0.02

# Filled by kernel() for test harness introspection.
last_stats: dict = {}


def _phase1_nc() -> bass.Bass:
    DT1 = mybir.dt.float16 if P1_F16 else F32
    nc = bacc.Bacc("TRN2", target_bir_lowering=False, num_devices=NC)
    xT = nc.dram_tensor("xT", [D, PC], DT1, kind="ExternalInput")
    pw = nc.dram_tensor("pw", [D, D], DT1, kind="ExternalInput")
    pb = nc.dram_tensor("pb", [128, DK], F32, kind="ExternalInput")
    rw = nc.dram_tensor("rw", [D, D], DT1, kind="ExternalInput")
    kT2 = nc.dram_tensor("kT2", [D, E], DT1, kind="ExternalInput")
    kk1 = nc.dram_tensor("kk1", [1, E], F32, kind="ExternalInput")
    onc = nc.dram_tensor("onc", [128, 1], DT1, kind="ExternalInput")
    onr = nc.dram_tensor("onr", [1, 512], F32, kind="ExternalInput")
    hT = nc.dram_tensor("hT", [D, PC], DT1, kind="ExternalOutput")
    d2T = nc.dram_tensor("d2T", [E, PC], F32, kind="ExternalOutput")

    NCH = PC // 512  # 512-column chunks

    with tile.TileContext(nc) as tc:
        with (
            tc.tile_pool(name="const", bufs=1) as cpool,
            tc.tile_pool(name="act", bufs=1) as apool,
            tc.tile_pool(name="ps", bufs=2, space="PSUM") as pspool,
            tc.tile_pool(name="ps_small", bufs=2, space="PSUM") as psmall,
        ):
            xT_sb = cpool.tile([128, DK, PC], DT1, tag="xT")
            pw_sb = cpool.tile([128, DK, D], DT1, tag="pw")
            rw_sb = cpool.tile([128, DK, D], DT1, tag="rw")
            for k in range(DK):
                nc.sync.dma_start(xT_sb[:, k, :], xT[k * 128 : (k + 1) * 128, :])
                nc.sync.dma_start(pw_sb[:, k, :], pw[k * 128 : (k + 1) * 128, :])
                nc.sync.dma_start(rw_sb[:, k, :], rw[k * 128 : (k + 1) * 128, :])
            pb_sb = cpool.tile([128, DK], F32, tag="pb")
            nc.sync.dma_start(pb_sb[:], pb[:])
            kT2_sb = cpool.tile([128, DK, E], DT1, tag="kT2")
            for k in range(DK):
                nc.sync.dma_start(kT2_sb[:, k, :], kT2[k * 128 : (k + 1) * 128, :])
            kk_sb = cpool.tile([1, E], F32, tag="kk")
            nc.sync.dma_start(kk_sb[:], kk1[:])
            onc_sb = cpool.tile([128, 1], DT1, tag="onc")
            nc.sync.dma_start(onc_sb[:], onc[:])
            onr_sb = cpool.tile([1, 512], F32, tag="onr")
            nc.sync.dma_start(onr_sb[:], onr[:])

            hT_sb = apool.tile([128, DK, PC], DT1, tag="hT")
            rT_sb = apool.tile([128, DK, PC], DT1, tag="rT")
            r2_sb = apool.tile([128, DK, PC], DT1, tag="r2")
            rr_sb = apool.tile([1, PC], F32, tag="rr")
            d2_sb = apool.tile([E, PC], F32, tag="d2")

            # h^T = pw^T-mm(x^T) + pb ; r^T = rw^T-mm(h^T)
            for w_sb, src, dst, bias in (
                (pw_sb, xT_sb, hT_sb, pb_sb),
                (rw_sb, hT_sb, rT_sb, None),
            ):
                for m in range(DK):
                    for n in range(NCH):
                        ps = pspool.tile([128, 512], F32, tag="ps")
                        for k in range(DK):
                            nc.tensor.matmul(
                                ps[:],
                                w_sb[:, k, m * 128 : (m + 1) * 128],
                                src[:, k, n * 512 : (n + 1) * 512],
                                start=(k == 0),
                                stop=(k == DK - 1),
                            )
                        if bias is not None:
                            nc.scalar.activation(
                                dst[:, m, n * 512 : (n + 1) * 512], ps[:],
                                AF.Identity, bias=bias[:, m : m + 1],
                            )
                        else:
                            nc.scalar.activation(
                                dst[:, m, n * 512 : (n + 1) * 512], ps[:], AF.Copy
                            )
            nc.sync.dma_start(hT.rearrange("(m p) n -> p m n", p=128), hT_sb[:])

            # rr = sum_d r^2 (ones-matmul over partition tiles)
            for kt in range(DK):
                nc.vector.tensor_mul(
                    r2_sb[:, kt, :], rT_sb[:, kt, :], rT_sb[:, kt, :]
                )
            for n in range(NCH):
                ps1 = psmall.tile([1, 512], F32, tag="ps1")
                for k in range(DK):
                    nc.tensor.matmul(
                        ps1[:], onc_sb[:], r2_sb[:, k, n * 512 : (n + 1) * 512],
                        start=(k == 0), stop=(k == DK - 1),
                    )
                nc.scalar.activation(rr_sb[:, n * 512 : (n + 1) * 512], ps1[:], AF.Copy)

            # d2 = (-2 keys) . r + |k|^2 + rr  (mixed fp16/fp32 psum group)
            for n in range(NCH):
                psA = psmall.tile([E, 512], F32, tag="psA")
                for k in range(DK):
                    nc.tensor.matmul(
                        psA[:], kT2_sb[:, k, :], rT_sb[:, k, n * 512 : (n + 1) * 512],
                        start=(k == 0), stop=False,
                    )
                nc.tensor.matmul(psA[:], kk_sb[:], onr_sb[:], start=False, stop=False)
                nc.tensor.matmul(
                    psA[:], onr_sb[:, 0:E], rr_sb[:, n * 512 : (n + 1) * 512],
                    start=False, stop=True,
                )
                nc.scalar.activation(d2_sb[:, n * 512 : (n + 1) * 512], psA[:], AF.Copy)
            nc.sync.dma_start(d2T[:], d2_sb[:])
    nc.compile()
    return nc


def _phase2_nc(S: int, DT2=None) -> bass.Bass:
    if DT2 is None:
        DT2 = mybir.dt.float16 if P2_F16 else F32R
    nc = bacc.Bacc("TRN2", target_bir_lowering=False, num_devices=NC)
    C = S * L
    hseg = nc.dram_tensor("hseg", [D, C], DT2, kind="ExternalInput")
    gseg = nc.dram_tensor("gseg", [128, C], F32, kind="ExternalInput")
    w1s = nc.dram_tensor("w1s", [S, D, H], DT2, kind="ExternalInput")
    w2s = nc.dram_tensor("w2s", [S, H, D], DT2, kind="ExternalInput")
    b1s = nc.dram_tensor("b1s", [128, S * HK], F32, kind="ExternalInput")
    b2s = nc.dram_tensor("b2s", [128, S * DK], F32, kind="ExternalInput")
    oseg = nc.dram_tensor("oseg", [D, C], F32, kind="ExternalOutput")

    with tile.TileContext(nc) as tc:
        with (
            tc.tile_pool(name="const", bufs=1) as cpool,
            tc.tile_pool(name="w1p", bufs=3) as w1p,
            tc.tile_pool(name="w2p", bufs=3) as w2p,
            tc.tile_pool(name="hp", bufs=3) as hp,
            tc.tile_pool(name="hidp", bufs=3) as hidp,
            tc.tile_pool(name="op", bufs=3) as op,
            tc.tile_pool(name="hid_ps", bufs=2, space="PSUM") as hidps,
            tc.tile_pool(name="out_ps", bufs=1, space="PSUM") as outps,
        ):
            gseg_sb = cpool.tile([128, C], F32, tag="gseg")
            b1_sb = cpool.tile([128, S * HK], F32, tag="b1")
            nc.sync.dma_start(b1_sb[:], b1s[:])
            b2_sb = cpool.tile([128, S * DK], F32, tag="b2")
            nc.sync.dma_start(b2_sb[:], b2s[:])

            for s in range(S):
                ht = hp.tile([128, DK, L], DT2, tag="h")
                for k in range(DK):
                    nc.sync.dma_start(
                        ht[:, k, :],
                        hseg[k * 128 : (k + 1) * 128, s * L : (s + 1) * L],
                    )
                w1t = w1p.tile([128, DK, H], DT2, tag="w1")
                for k in range(DK):
                    nc.sync.dma_start(
                        w1t[:, k, :], w1s[s, k * 128 : (k + 1) * 128, :]
                    )
                w2t = w2p.tile([128, HK, D], DT2, tag="w2")
                for j in range(DK):
                    nc.sync.dma_start(
                        w2t[:, 4 * j : 4 * j + 4, :],
                        w2s[s, 4 * j * 128 : (4 * j + 4) * 128, :].rearrange(
                            "(k p) d -> p k d", p=128
                        ),
                    )
                nc.sync.dma_start(
                    gseg_sb[:, s * L : (s + 1) * L], gseg[:, s * L : (s + 1) * L]
                )
                opsum = outps.tile([128, DK, L], F32, tag="opsum", name=f"opsum_{s}")
                ops = [opsum[:, mo, :] for mo in range(DK)]
                for m in range(HK):
                    hps = hidps.tile([128, L], F32, tag="hps")
                    for k in range(DK):
                        nc.tensor.matmul(
                            hps[:],
                            w1t[:, k, m * 128 : (m + 1) * 128],
                            ht[:, k, :],
                            start=(k == 0),
                            stop=(k == DK - 1),
                        )
                    hidt = hidp.tile([128, L], DT2, tag="hid")
                    nc.scalar.activation(
                        hidt[:], hps[:], AF.Gelu,
                        bias=b1_sb[:, s * HK + m : s * HK + m + 1],
                    )
                    for mo in range(DK):
                        nc.tensor.matmul(
                            ops[mo][:],
                            w2t[:, m, mo * 128 : (mo + 1) * 128],
                            hidt[:],
                            start=(m == 0),
                            stop=(m == HK - 1),
                        )
                for mo in range(DK):
                    ot = op.tile([128, L], F32, tag="o")
                    nc.vector.scalar_tensor_tensor(
                        ot[:],
                        ops[mo][:],
                        b2_sb[:, s * DK + mo : s * DK + mo + 1],
                        gseg_sb[:, s * L : (s + 1) * L],
                        ALU.add,
                        ALU.mult,
                    )
                    nc.sync.dma_start(
                        oseg[mo * 128 : (mo + 1) * 128, s * L : (s + 1) * L], ot[:]
                    )
    nc.compile()
    return nc


def _run(nc, in_maps, label):
    trace = os.environ.get("KTRACE") == "1"
    res = run_bass_kernel_spmd(
        nc, in_maps, core_ids=list(range(NC)), trace=trace
    )
    if trace:
        last_stats[label] = {
            "exec_time_ns": res.exec_time_ns,
            "mean_exec_time_ns": res.mean_exec_time_ns,
            "trace": res.instructions_and_trace[1]
            if res.instructions_and_trace
            else None,
        }
    return res.results


def kernel(view0, view1, proj_w, proj_b, router_w, expert_keys, w1, b1, w2, b2):
    view0 = np.ascontiguousarray(view0, dtype=np.float32)
    view1 = np.ascontiguousarray(view1, dtype=np.float32)
    proj_w = np.asarray(proj_w, dtype=np.float32)
    proj_b = np.asarray(proj_b, dtype=np.float32)
    router_w = np.asarray(router_w, dtype=np.float32)
    keys = np.asarray(expert_keys, dtype=np.float32)
    w1 = np.asarray(w1, dtype=np.float32)
    b1 = np.asarray(b1, dtype=np.float32)
    w2 = np.asarray(w2, dtype=np.float32)
    b2 = np.asarray(b2, dtype=np.float32)

    # ---- Phase 1: h and d2 on device (token-parallel over 8 cores) ----
    dt1 = np.float16 if P1_F16 else np.float32
    xT_full = np.concatenate(
        [view0.reshape(N, D).T, view1.reshape(N, D).T], axis=1
    )  # [D, NT], column t = view*N + (b*T + tt)
    xT_d = np.ascontiguousarray(xT_full, dtype=dt1)

    kT2 = np.ascontiguousarray(-2.0 * keys.T).astype(dt1)  # [D, E]
    kk1 = (keys * keys).sum(axis=1, dtype=np.float32).reshape(1, E)
    onc = np.ones((128, 1), dt1)
    onr = np.ones((1, 512), np.float32)

    in_maps1 = []
    for c in range(NC):
        v = (c * PC) // N  # cores 0-3 -> view 0, 4-7 -> view 1
        pb_t = np.ascontiguousarray(proj_b[v].reshape(DK, 128).T)  # [128, DK]
        in_maps1.append(
            {
                "xT": np.ascontiguousarray(xT_d[:, c * PC : (c + 1) * PC]),
                "pw": np.ascontiguousarray(proj_w[v], dtype=dt1),
                "pb": pb_t,
                "rw": np.ascontiguousarray(router_w[v], dtype=dt1),
                "kT2": kT2,
                "kk1": kk1,
                "onc": onc,
                "onr": onr,
            }
        )
    res1 = _run(_phase1_nc(), in_maps1, "phase1")

    hT_full = np.concatenate([r["hT"] for r in res1], axis=1)  # [D, NT], dt1
    d2 = np.concatenate([r["d2T"] for r in res1], axis=1).T   # [NT, E] fp32

    # ---- Host repair: recompute borderline tokens exactly in fp32 ----
    if P1_F16:
        logits0 = -np.sqrt(np.maximum(d2, 0.0), dtype=np.float32)
        part = np.partition(logits0, E - K - 1, axis=1)
        gap45 = part[:, E - K] - part[:, E - K - 1]  # 4th minus 5th logit
        risk = np.nonzero(gap45 < REPAIR_MARGIN)[0]
        last_stats["n_repaired"] = int(risk.size)
        if risk.size:
            x_all = np.concatenate(
                [view0.reshape(N, D), view1.reshape(N, D)], axis=0
            )
            vsel = (risk >= N).astype(np.int64)
            kkr = kk1.reshape(E)
            for v in (0, 1):
                rt = risk[vsel == v]
                if rt.size == 0:
                    continue
                hx = x_all[rt] @ proj_w[v] + proj_b[v]
                rx = hx @ router_w[v]
                d2[rt] = (
                    (rx * rx).sum(axis=1, keepdims=True)
                    - 2.0 * (rx @ keys.T)
                    + kkr
                )

    # ---- Host routing: logits, top-4, softmax gates (fp32) ----
    logits = -np.sqrt(np.maximum(d2, 0.0), dtype=np.float32)
    topi = np.argsort(-logits, axis=1, kind="stable")[:, :K]   # [NT, K]
    topv = np.take_along_axis(logits, topi, axis=1)
    ex = np.exp(topv - topv[:, :1], dtype=np.float32)
    gates = ex / ex.sum(axis=1, keepdims=True, dtype=np.float32)

    # ---- Slot plan: per expert, tokens cut into L-column slots ----
    slots = []  # (expert, token_ids, gate_vals)
    for e in range(E):
        sel_tok, sel_k = np.nonzero(topi == e)
        if sel_tok.size == 0:
            continue
        g_e = gates[sel_tok, sel_k]
        for i in range(0, sel_tok.size, L):
            slots.append((e, sel_tok[i : i + L], g_e[i : i + L]))
    S = max(1, math.ceil(len(slots) / NC))
    while len(slots) < S * NC:
        slots.append((-1, np.zeros(0, np.int64), np.zeros(0, np.float32)))

    # ---- Phase 2 inputs ----
    C = S * L
    dt2 = np.float16 if P2_F16 else np.float32
    w1_d = w1.astype(dt2)
    w2_d = w2.astype(dt2)
    hT_d = hT_full.astype(dt2)
    in_maps2 = []
    core_slots = []
    for c in range(NC):
        csl = slots[c * S : (c + 1) * S]
        core_slots.append(csl)
        hseg = np.zeros((D, C), dt2)
        gseg = np.zeros((1, C), np.float32)
        w1c = np.zeros((S, D, H), dt2)
        w2c = np.zeros((S, H, D), dt2)
        b1c = np.zeros((128, S * HK), np.float32)
        b2c = np.zeros((128, S * DK), np.float32)
        for s, (e, toks, gv) in enumerate(csl):
            if e < 0:
                continue
            n = toks.size
            hseg[:, s * L : s * L + n] = hT_d[:, toks]
            gseg[0, s * L : s * L + n] = gv
            w1c[s] = w1_d[e]
            w2c[s] = w2_d[e]
            b1c[:, s * HK : (s + 1) * HK] = b1[e].reshape(HK, 128).T
            b2c[:, s * DK : (s + 1) * DK] = b2[e].reshape(DK, 128).T
        in_maps2.append(
            {
                "hseg": hseg,
                "gseg": np.ascontiguousarray(
                    np.broadcast_to(gseg, (128, C))
                ),
                "w1s": w1c,
                "w2s": w2c,
                "b1s": b1c,
                "b2s": b2c,
            }
        )
    last_stats["S"] = S
    last_stats["n_slots_real"] = sum(
        1 for e, _, _ in slots if e >= 0
    )
    res2 = _run(_phase2_nc(S), in_maps2, "phase2")

    # ---- Combine ----
    fusedT = np.zeros((D, NT), np.float32)
    for c in range(NC):
        o = res2[c]["oseg"]  # [D, C]
        for s, (e, toks, _gv) in enumerate(core_slots[c]):
            if e < 0 or toks.size == 0:
                continue
            fusedT[:, toks] += o[:, s * L : s * L + toks.size]
    fused = (fusedT[:, :N] + fusedT[:, N:]).T  # [N, D]
    return np.ascontiguousarray(fused.reshape(B, T, D), dtype=np.float32)



# revision 2
# speedup vs baseline: 1.1039x; 1.1039x over previous
"""Trainium2 Bass kernel for nn_MoEElementFusion (moe_routing).

Strategy (8 NeuronCores, SPMD, two launches with host routing in between):
  Phase 1 (token-data-parallel): each core takes 1/8 of the 8192 (view,token)
  columns, computes in fp16 on the PE (psum fp32):
      h   = x @ proj_w + proj_b
      r   = h @ router_w
      d2X = (-2 keys) . r        and   rr = |r|^2
  The PE is pre-warmed with a 12-matmul accumulation on a zeros tile so the
  HAM clock-gate releases (1.2 -> 2.4 GHz) while the inputs stream in over
  both HWDGE queues (sync + scalar). h^T chunks store as produced (gpsimd
  SWDGE). Host adds |k|^2 + rr, repairs borderline top-4 rows in fp32,
  takes stable top-4 and softmax gates.

  Slot plan: per expert, selected columns cut into full-512 pieces plus a
  remainder. A small search promotes the largest remainders into full cells
  and groups the rest into short tail slots, minimizing the per-core column
  total (SPMD: every core runs the same compile-time slot-length list,
  which expert fills each slot is pure input data).

  Phase 2 (compiled at runtime once the length list is known): per slot,
  FFN in fp16 (1 cycle/row):
      out^T = w2^T-mm(gelu(w1^T-mm(h^T) + b1)) + b2
  Weights double/triple-buffered, streamed over both HWDGE queues; outputs
  drained from PSUM by ACT+DVE in parallel and written via gpsimd SWDGE so
  the sync queue never head-of-line blocks next-slot weight loads. Gates
  are applied on the host during the combine.
"""

import math
import os

import numpy as np

import concourse.bass as bass
import concourse.bacc as bacc
import concourse.mybir as mybir
import concourse.tile as tile
from concourse.bass_utils import run_bass_kernel_spmd

# Problem dims (hardcoded per spec)
V, B, T, D, E, K = 2, 4, 1024, 512, 16, 4
H = 4 * D
N = B * T          # tokens per view
NT = V * N         # total (view, token) columns = 8192
NC = 8             # cores
PC = NT // NC      # phase-1 columns per core = 1024

F16 = mybir.dt.float16
F32 = mybir.dt.float32
AF = mybir.ActivationFunctionType
ALU = mybir.AluOpType

DK = D // 128      # 4 k-tiles over D
HK = H // 128      # 16 k-tiles over H

REPAIR_MARGIN = 0.02
NWARM = 12         # warm-up matmuls (~5us busy: releases the HAM clock gate)
MIN_SLOT = 192     # shorter slots would bottleneck on ACT-engine work

# Filled by kernel() for test harness introspection.
last_stats: dict = {}


def _warmup(nc, tc, cpool, pspool, wz, wout, ps_tag):
    """12 chained matmuls on a zeros tile accumulating into one PSUM bank,
    drained to a dummy output (not DCE-able). Keeps the PE busy ~5us from
    t~0.5us so HAM unthrottles while real inputs stream in."""
    wz_sb = cpool.tile([128, 512], F16, tag="wz")
    nc.sync.dma_start(wz_sb[:], wz[:])
    wps = pspool.tile([128, 512], F32, tag=ps_tag)
    for i in range(NWARM):
        nc.tensor.matmul(
            wps[:], wz_sb[:, 0:128], wz_sb[:],
            start=(i == 0), stop=(i == NWARM - 1),
        )
    wo_sb = cpool.tile([128, 512], F32, tag="wo")
    nc.scalar.activation(wo_sb[:], wps[:], AF.Copy)
    nc.gpsimd.dma_start(wout[:], wo_sb[:])


def _phase1_nc() -> bass.Bass:
    nc = bacc.Bacc("TRN2", target_bir_lowering=False, num_devices=NC)
    xT = nc.dram_tensor("xT", [D, PC], F16, kind="ExternalInput")
    pw = nc.dram_tensor("pw", [D, D], F16, kind="ExternalInput")
    pb = nc.dram_tensor("pb", [128, DK], F32, kind="ExternalInput")
    rw = nc.dram_tensor("rw", [D, D], F16, kind="ExternalInput")
    kT2 = nc.dram_tensor("kT2", [D, E], F16, kind="ExternalInput")
    onc = nc.dram_tensor("onc", [128, 1], F16, kind="ExternalInput")
    wz = nc.dram_tensor("wz", [128, 512], F16, kind="ExternalInput")
    hT = nc.dram_tensor("hT", [D, PC], F16, kind="ExternalOutput")
    d2X = nc.dram_tensor("d2X", [E, PC], F32, kind="ExternalOutput")
    rrO = nc.dram_tensor("rrO", [1, PC], F32, kind="ExternalOutput")
    wout = nc.dram_tensor("wout", [128, 512], F32, kind="ExternalOutput")

    NCH = PC // 512  # 512-column compute chunks

    with tile.TileContext(nc) as tc:
        with (
            tc.tile_pool(name="const", bufs=1) as cpool,
            tc.tile_pool(name="act", bufs=1) as apool,
            tc.tile_pool(name="ps", bufs=2, space="PSUM") as pspool,
            tc.tile_pool(name="ps_small", bufs=2, space="PSUM") as psmall,
        ):
            _warmup(nc, tc, cpool, pspool, wz, wout, "ps")

            # Inputs: xT in 256-col chunks, weights per k-tile, spread over
            # the two HWDGE trigger queues (sync=SP, scalar=ACT).
            pw_sb = cpool.tile([128, DK, D], F16, tag="pw")
            for k in range(DK):
                eng = nc.scalar if k % 2 else nc.sync
                eng.dma_start(pw_sb[:, k, :], pw[k * 128 : (k + 1) * 128, :])
            xT_sb = cpool.tile([128, DK, PC], F16, tag="xT")
            for n in range(PC // 256):
                for k in range(DK):
                    eng = nc.scalar if (n * DK + k) % 2 else nc.sync
                    eng.dma_start(
                        xT_sb[:, k, n * 256 : (n + 1) * 256],
                        xT[k * 128 : (k + 1) * 128, n * 256 : (n + 1) * 256],
                    )
            rw_sb = cpool.tile([128, DK, D], F16, tag="rw")
            for k in range(DK):
                eng = nc.scalar if k % 2 else nc.sync
                eng.dma_start(rw_sb[:, k, :], rw[k * 128 : (k + 1) * 128, :])
            pb_sb = cpool.tile([128, DK], F32, tag="pb")
            nc.scalar.dma_start(pb_sb[:], pb[:])
            kT2_sb = cpool.tile([128, DK, E], F16, tag="kT2")
            for k in range(DK):
                nc.scalar.dma_start(kT2_sb[:, k, :], kT2[k * 128 : (k + 1) * 128, :])
            onc_sb = cpool.tile([128, 1], F16, tag="onc")
            nc.scalar.dma_start(onc_sb[:], onc[:])

            hT_sb = apool.tile([128, DK, PC], F16, tag="hT")
            rT_sb = apool.tile([128, DK, PC], F16, tag="rT")
            r2_sb = apool.tile([128, DK, PC], F16, tag="r2")
            rr_sb = apool.tile([1, PC], F32, tag="rr")
            d2_sb = apool.tile([E, PC], F32, tag="d2")

            # h^T = pw^T-mm(x^T) + pb ; r^T = rw^T-mm(h^T)
            for w_sb, src, dst, bias in (
                (pw_sb, xT_sb, hT_sb, pb_sb),
                (rw_sb, hT_sb, rT_sb, None),
            ):
                for m in range(DK):
                    for n in range(NCH):
                        ps = pspool.tile([128, 512], F32, tag="ps")
                        for k in range(DK):
                            nc.tensor.matmul(
                                ps[:],
                                w_sb[:, k, m * 128 : (m + 1) * 128],
                                src[:, k, n * 512 : (n + 1) * 512],
                                start=(k == 0),
                                stop=(k == DK - 1),
                            )
                        if bias is not None:
                            nc.scalar.activation(
                                dst[:, m, n * 512 : (n + 1) * 512], ps[:],
                                AF.Identity, bias=bias[:, m : m + 1],
                            )
                            nc.gpsimd.dma_start(
                                hT[m * 128 : (m + 1) * 128,
                                   n * 512 : (n + 1) * 512],
                                dst[:, m, n * 512 : (n + 1) * 512],
                            )
                        else:
                            nc.scalar.activation(
                                dst[:, m, n * 512 : (n + 1) * 512], ps[:], AF.Copy
                            )

            # rr = sum_d r^2 (ones-matmul over partition tiles)
            for kt in range(DK):
                nc.vector.tensor_mul(
                    r2_sb[:, kt, :], rT_sb[:, kt, :], rT_sb[:, kt, :]
                )
            for n in range(NCH):
                ps1 = psmall.tile([1, 512], F32, tag="ps1")
                for k in range(DK):
                    nc.tensor.matmul(
                        ps1[:], onc_sb[:], r2_sb[:, k, n * 512 : (n + 1) * 512],
                        start=(k == 0), stop=(k == DK - 1),
                    )
                nc.scalar.activation(rr_sb[:, n * 512 : (n + 1) * 512], ps1[:], AF.Copy)
            nc.gpsimd.dma_start(rrO[:], rr_sb[:])

            # d2X = (-2 keys) . r   (|k|^2 and rr are added on the host)
            for n in range(NCH):
                psA = psmall.tile([E, 512], F32, tag="psA")
                for k in range(DK):
                    nc.tensor.matmul(
                        psA[:], kT2_sb[:, k, :], rT_sb[:, k, n * 512 : (n + 1) * 512],
                        start=(k == 0), stop=(k == DK - 1),
                    )
                nc.scalar.activation(d2_sb[:, n * 512 : (n + 1) * 512], psA[:], AF.Copy)
            nc.sync.dma_start(d2X[:], d2_sb[:])
    nc.compile()
    return nc


def _phase2_nc(lens: tuple) -> bass.Bass:
    S = len(lens)
    offs = [0]
    for L in lens:
        offs.append(offs[-1] + L)
    Ctot = offs[-1]

    nc = bacc.Bacc("TRN2", target_bir_lowering=False, num_devices=NC)
    hseg = nc.dram_tensor("hseg", [D, Ctot], F16, kind="ExternalInput")
    w1s = nc.dram_tensor("w1s", [S, D, H], F16, kind="ExternalInput")
    w2s = nc.dram_tensor("w2s", [S, H, D], F16, kind="ExternalInput")
    b1s = nc.dram_tensor("b1s", [128, S * HK], F32, kind="ExternalInput")
    b2s = nc.dram_tensor("b2s", [128, S * DK], F32, kind="ExternalInput")
    wz = nc.dram_tensor("wz", [128, 512], F16, kind="ExternalInput")
    oseg = nc.dram_tensor("oseg", [D, Ctot], F32, kind="ExternalOutput")
    wout = nc.dram_tensor("wout", [128, 512], F32, kind="ExternalOutput")

    with tile.TileContext(nc) as tc:
        with (
            tc.tile_pool(name="const", bufs=1) as cpool,
            tc.tile_pool(name="w1p", bufs=3) as w1p,
            tc.tile_pool(name="w2p", bufs=3) as w2p,
            tc.tile_pool(name="hp", bufs=3) as hp,
            tc.tile_pool(name="hidp", bufs=3) as hidp,
            tc.tile_pool(name="op", bufs=4) as op,
            tc.tile_pool(name="hid_ps", bufs=2, space="PSUM") as hidps,
            tc.tile_pool(name="out_ps", bufs=1, space="PSUM") as outps,
        ):
            _warmup(nc, tc, cpool, hidps, wz, wout, "hps")

            b1_sb = cpool.tile([128, S * HK], F32, tag="b1")
            nc.scalar.dma_start(b1_sb[:], b1s[:])
            b2_sb = cpool.tile([128, S * DK], F32, tag="b2")
            nc.scalar.dma_start(b2_sb[:], b2s[:])

            for s in range(S):
                Lc = lens[s]
                off = offs[s]
                ht = hp.tile([128, DK, 512], F16, tag="h")
                for k in range(DK):
                    nc.sync.dma_start(
                        ht[:, k, :Lc],
                        hseg[k * 128 : (k + 1) * 128, off : off + Lc],
                    )
                w1t = w1p.tile([128, DK, H], F16, tag="w1")
                for k in range(DK):
                    eng = nc.scalar if k % 2 else nc.sync
                    eng.dma_start(w1t[:, k, :], w1s[s, k * 128 : (k + 1) * 128, :])
                w2t = w2p.tile([128, HK, D], F16, tag="w2")
                for j in range(DK):
                    eng = nc.scalar if j % 2 else nc.sync
                    eng.dma_start(
                        w2t[:, 4 * j : 4 * j + 4, :],
                        w2s[s, 4 * j * 128 : (4 * j + 4) * 128, :].rearrange(
                            "(k p) d -> p k d", p=128
                        ),
                    )
                opsum = outps.tile([128, DK, 512], F32, tag="opsum", name=f"opsum_{s}")
                for m in range(HK):
                    hps = hidps.tile([128, 512], F32, tag="hps")
                    for k in range(DK):
                        nc.tensor.matmul(
                            hps[:, :Lc],
                            w1t[:, k, m * 128 : (m + 1) * 128],
                            ht[:, k, :Lc],
                            start=(k == 0),
                            stop=(k == DK - 1),
                        )
                    hidt = hidp.tile([128, 512], F16, tag="hid")
                    nc.scalar.activation(
                        hidt[:, :Lc], hps[:, :Lc], AF.Gelu,
                        bias=b1_sb[:, s * HK + m : s * HK + m + 1],
                    )
                    for mo in range(DK):
                        nc.tensor.matmul(
                            opsum[:, mo, :Lc],
                            w2t[:, m, mo * 128 : (mo + 1) * 128],
                            hidt[:, :Lc],
                            start=(m == 0),
                            stop=(m == HK - 1),
                        )
                # Drain PSUM with ACT+DVE in parallel; write via gpsimd SWDGE
                # so the sync HWDGE queue stays free for next-slot weights.
                for mo in range(DK):
                    ot = op.tile([128, 512], F32, tag="o")
                    bcol = b2_sb[:, s * DK + mo : s * DK + mo + 1]
                    if mo % 2:
                        nc.vector.tensor_scalar(
                            ot[:, :Lc], opsum[:, mo, :Lc], bcol, None, ALU.add
                        )
                    else:
                        nc.scalar.activation(
                            ot[:, :Lc], opsum[:, mo, :Lc], AF.Identity, bias=bcol
                        )
                    nc.gpsimd.dma_start(
                        oseg[mo * 128 : (mo + 1) * 128, off : off + Lc],
                        ot[:, :Lc],
                    )
    nc.compile()
    return nc


def _run(nc, in_maps, label):
    trace = os.environ.get("KTRACE") == "1"
    res = run_bass_kernel_spmd(
        nc, in_maps, core_ids=list(range(NC)), trace=trace
    )
    if trace:
        last_stats[label] = {
            "exec_time_ns": res.exec_time_ns,
            "mean_exec_time_ns": res.mean_exec_time_ns,
            "trace": res.instructions_and_trace[1]
            if res.instructions_and_trace
            else None,
        }
    return res.results


def _round16(x: int) -> int:
    return (x + 15) & ~15


def _plan_slots(topi, gates):
    """Cut each expert's selected columns into pieces and pick compile-time
    slot lengths minimizing the per-core column total (SPMD-uniform)."""
    per_e = []  # (expert, token_ids, gate_vals)
    for e in range(E):
        sel_tok, sel_k = np.nonzero(topi == e)
        per_e.append((sel_tok, gates[sel_tok, sel_k]))

    full_pieces = []  # (e, start)
    rem_pieces = []   # (e, start, len)
    for e in range(E):
        n = per_e[e][0].size
        f, r = divmod(n, 512)
        for i in range(f):
            full_pieces.append((e, i * 512, 512))
        if r:
            rem_pieces.append((e, f * 512, r))
    rem_pieces.sort(key=lambda p: -p[2])

    Fn = len(full_pieces)
    best = None
    for p in range(len(rem_pieces) + 1):
        n512 = Fn + p
        S512 = max(1, math.ceil(n512 / NC))
        rest = rem_pieces[p:]
        tlens = [
            max(MIN_SLOT, _round16(rest[i * NC][2]))
            for i in range(math.ceil(len(rest) / NC))
        ]
        cost = 512 * S512 + sum(tlens)
        if best is None or cost < best[0]:
            best = (cost, p, S512, tlens)
    _, p, S512, tlens = best

    # cells[s][c] = (expert, start, n) or None
    full_cells = full_pieces + rem_pieces[:p]
    full_cells += [None] * (S512 * NC - len(full_cells))
    rest = rem_pieces[p:]
    slots = []  # (length, [8 cells])
    for s in range(S512):
        slots.append((512, full_cells[s * NC : (s + 1) * NC]))
    for i, tl in enumerate(tlens):
        cells = rest[i * NC : (i + 1) * NC]
        cells += [None] * (NC - len(cells))
        slots.append((tl, cells))

    # Interleave short slots between full ones (keeps weight prefetch ahead).
    full_slots = [sl for sl in slots if sl[0] == 512]
    tail_slots = [sl for sl in slots if sl[0] != 512]
    ordered = []
    ti = 0
    for i, sl in enumerate(full_slots):
        ordered.append(sl)
        if i % 2 == 1 and ti < len(tail_slots):
            ordered.append(tail_slots[ti])
            ti += 1
    ordered.extend(tail_slots[ti:])
    return per_e, ordered


def kernel(view0, view1, proj_w, proj_b, router_w, expert_keys, w1, b1, w2, b2):
    view0 = np.ascontiguousarray(view0, dtype=np.float32)
    view1 = np.ascontiguousarray(view1, dtype=np.float32)
    proj_w = np.asarray(proj_w, dtype=np.float32)
    proj_b = np.asarray(proj_b, dtype=np.float32)
    router_w = np.asarray(router_w, dtype=np.float32)
    keys = np.asarray(expert_keys, dtype=np.float32)
    w1 = np.asarray(w1, dtype=np.float32)
    b1 = np.asarray(b1, dtype=np.float32)
    w2 = np.asarray(w2, dtype=np.float32)
    b2 = np.asarray(b2, dtype=np.float32)

    # ---- Phase 1: h, cross-term d2X and rr on device ----
    xT_full = np.concatenate(
        [view0.reshape(N, D).T, view1.reshape(N, D).T], axis=1
    )  # [D, NT], column t = view*N + (b*T + tt)
    xT_d = np.ascontiguousarray(xT_full, dtype=np.float16)

    kT2 = np.ascontiguousarray(-2.0 * keys.T).astype(np.float16)  # [D, E]
    kk = (keys * keys).sum(axis=1, dtype=np.float32)  # [E]
    onc = np.ones((128, 1), np.float16)
    wz = np.zeros((128, 512), np.float16)

    in_maps1 = []
    for c in range(NC):
        v = (c * PC) // N  # cores 0-3 -> view 0, 4-7 -> view 1
        pb_t = np.ascontiguousarray(proj_b[v].reshape(DK, 128).T)  # [128, DK]
        in_maps1.append(
            {
                "xT": np.ascontiguousarray(xT_d[:, c * PC : (c + 1) * PC]),
                "pw": np.ascontiguousarray(proj_w[v], dtype=np.float16),
                "pb": pb_t,
                "rw": np.ascontiguousarray(router_w[v], dtype=np.float16),
                "kT2": kT2,
                "onc": onc,
                "wz": wz,
            }
        )
    res1 = _run(_phase1_nc(), in_maps1, "phase1")

    hT_d = np.concatenate([r["hT"] for r in res1], axis=1)       # [D, NT] f16
    d2 = np.concatenate([r["d2X"] for r in res1], axis=1).T      # [NT, E] f32
    rr = np.concatenate([r["rrO"] for r in res1], axis=1).T      # [NT, 1] f32
    d2 += rr
    d2 += kk[None, :]

    # ---- Host repair: recompute borderline tokens exactly in fp32 ----
    logits0 = -np.sqrt(np.maximum(d2, 0.0), dtype=np.float32)
    part = np.partition(logits0, E - K - 1, axis=1)
    gap45 = part[:, E - K] - part[:, E - K - 1]  # 4th minus 5th logit
    risk = np.nonzero(gap45 < REPAIR_MARGIN)[0]
    last_stats["n_repaired"] = int(risk.size)
    if risk.size:
        x_all = np.concatenate(
            [view0.reshape(N, D), view1.reshape(N, D)], axis=0
        )
        vsel = (risk >= N).astype(np.int64)
        for v in (0, 1):
            rt = risk[vsel == v]
            if rt.size == 0:
                continue
            hx = x_all[rt] @ proj_w[v] + proj_b[v]
            rx = hx @ router_w[v]
            d2[rt] = (
                (rx * rx).sum(axis=1, keepdims=True)
                - 2.0 * (rx @ keys.T)
                + kk
            )

    # ---- Host routing: logits, top-4, softmax gates (fp32) ----
    logits = -np.sqrt(np.maximum(d2, 0.0), dtype=np.float32)
    topi = np.argsort(-logits, axis=1, kind="stable")[:, :K]   # [NT, K]
    topv = np.take_along_axis(logits, topi, axis=1)
    ex = np.exp(topv - topv[:, :1], dtype=np.float32)
    gates = ex / ex.sum(axis=1, keepdims=True, dtype=np.float32)

    # ---- Slot plan ----
    per_e, slots = _plan_slots(topi, gates)
    lens = tuple(sl[0] for sl in slots)
    S = len(lens)
    offs = np.concatenate([[0], np.cumsum(lens)]).astype(np.int64)
    Ctot = int(offs[-1])

    # ---- Phase 2 inputs ----
    w1_d = w1.astype(np.float16)
    w2_d = w2.astype(np.float16)
    in_maps2 = []
    core_cells = []  # per core: list over slots of (e, toks, gvals) or None
    for c in range(NC):
        hseg = np.zeros((D, Ctot), np.float16)
        w1c = np.zeros((S, D, H), np.float16)
        w2c = np.zeros((S, H, D), np.float16)
        b1c = np.zeros((128, S * HK), np.float32)
        b2c = np.zeros((128, S * DK), np.float32)
        cells = []
        for s, (Lc, cell8) in enumerate(slots):
            cell = cell8[c]
            if cell is None:
                cells.append(None)
                continue
            e, start, n = cell
            toks = per_e[e][0][start : start + n]
            gv = per_e[e][1][start : start + n]
            cells.append((e, toks, gv))
            hseg[:, offs[s] : offs[s] + n] = hT_d[:, toks]
            w1c[s] = w1_d[e]
            w2c[s] = w2_d[e]
            b1c[:, s * HK : (s + 1) * HK] = b1[e].reshape(HK, 128).T
            b2c[:, s * DK : (s + 1) * DK] = b2[e].reshape(DK, 128).T
        core_cells.append(cells)
        in_maps2.append(
            {
                "hseg": hseg,
                "w1s": w1c,
                "w2s": w2c,
                "b1s": b1c,
                "b2s": b2c,
                "wz": wz,
            }
        )
    last_stats["S"] = S
    last_stats["n_slots_real"] = sum(
        1 for cells in core_cells for cl in cells if cl is not None
    )
    last_stats["cols_per_core"] = Ctot
    res2 = _run(_phase2_nc(lens), in_maps2, "phase2")

    # ---- Combine (gates applied here) ----
    fusedT = np.zeros((D, NT), np.float32)
    for c in range(NC):
        o = res2[c]["oseg"]  # [D, Ctot]
        for s in range(S):
            cell = core_cells[c][s]
            if cell is None:
                continue
            e, toks, gv = cell
            n = toks.size
            fusedT[:, toks] += o[:, offs[s] : offs[s] + n] * gv[None, :]
    fused = (fusedT[:, :N] + fusedT[:, N:]).T  # [N, D]
    return np.ascontiguousarray(fused.reshape(B, T, D), dtype=np.float32)


# revision 5
# speedup vs baseline: 1.1739x; 1.0634x over previous
"""Trainium2 Bass kernel for nn_MoEElementFusion (moe_routing).

Strategy (8 NeuronCores, SPMD, two launches with host routing in between):
  Phase 1 (token-data-parallel): each core takes 1/8 of the 8192 (view,token)
  columns, computes in fp16 on the PE (psum fp32):
      h   = x @ proj_w + proj_b
      r   = h @ router_w
      d2X = (-2 keys) . r        and   rr = |r|^2
  Host adds |k|^2 + rr, repairs borderline top-4 rows in fp32, takes stable
  top-4 and softmax gates.

  Slot plan: per expert, selected columns cut into full-512 pieces plus a
  remainder; a small search promotes the largest remainders into full cells
  and groups the rest into short tail slots, minimizing the per-core column
  total (SPMD: every core runs the same compile-time slot-length list; which
  expert fills each slot is pure input data).

  Phase 2 (compiled at runtime once the length list is known): per slot,
  FFN in fp16 (1 cycle/row on the PE):
      out^T = w2^T-mm(gelu(w1^T-mm(h^T) + b1)) + b2

  Perf notes (from NTFF traces):
  - Every large DRAM tensor is host-repacked to partition-major layout so
    DMA descriptor lines are 4-16KB: HWDGE queues are descriptor-rate
    limited (~90 GB/s at 1KB lines), not bandwidth limited.
  - Each weight load is split across both HWDGE trigger queues (sync=SP,
    scalar=ACT); output writes go via gpsimd SWDGE + HWDGE so they never
    head-of-line block next-slot weight loads.
  - The PE is pre-warmed with a memset-fed matmul accumulation chain (no
    DMA dependency) so the HAM clock-gate releases (1.2 -> 2.4 GHz) while
    the first inputs stream in.
  - Gates are applied on the host during the combine.
"""

import math
import os

import numpy as np

import concourse.bass as bass
import concourse.bacc as bacc
import concourse.mybir as mybir
import concourse.tile as tile
from concourse.bass_utils import run_bass_kernel_spmd

# Problem dims (hardcoded per spec)
V, B, T, D, E, K = 2, 4, 1024, 512, 16, 4
H = 4 * D
N = B * T          # tokens per view
NT = V * N         # total (view, token) columns = 8192
NC = 8             # cores
PC = NT // NC      # phase-1 columns per core = 1024

F16 = mybir.dt.float16
F32 = mybir.dt.float32
AF = mybir.ActivationFunctionType
ALU = mybir.AluOpType

DK = D // 128      # 4 k-tiles over D
HK = H // 128      # 16 k-tiles over H

REPAIR_MARGIN = 0.02
NWARM1 = 24        # phase-1 warmup matmuls
NWARM2 = 30        # phase-2 warmup matmuls
MIN_SLOT = 192     # shorter slots would bottleneck on ACT-engine work

# Filled by kernel() for test harness introspection.
last_stats: dict = {}


def _warmup(nc, cpool, pspool, wout, ps_tag, nwarm):
    """Matmul chain on a memset tile accumulating into one PSUM bank, drained
    to a dummy output (not DCE-able). No input-DMA dependency: keeps the PE
    busy from ~4us so the HAM clock gate releases while inputs stream in."""
    wz_sb = cpool.tile([128, 512], F16, tag="wz")
    nc.vector.memset(wz_sb[:], 0.0)
    wps = pspool.tile([128, 512], F32, tag=ps_tag)
    for i in range(nwarm):
        nc.tensor.matmul(
            wps[:], wz_sb[:, 0:128], wz_sb[:],
            start=(i == 0), stop=(i == nwarm - 1),
        )
    wo_sb = cpool.tile([128, 512], F32, tag="wo")
    nc.scalar.activation(wo_sb[:], wps[:], AF.Copy)
    nc.gpsimd.dma_start(wout[:], wo_sb[:])


def _phase1_nc() -> bass.Bass:
    nc = bacc.Bacc("TRN2", target_bir_lowering=False, num_devices=NC)
    # Partition-major packed inputs (see host-side packing in kernel()).
    xT = nc.dram_tensor("xT", [128, DK * PC], F16, kind="ExternalInput")
    pw = nc.dram_tensor("pw", [128, DK * D], F16, kind="ExternalInput")
    pb = nc.dram_tensor("pb", [128, DK], F32, kind="ExternalInput")
    rw = nc.dram_tensor("rw", [128, DK * D], F16, kind="ExternalInput")
    kT2 = nc.dram_tensor("kT2", [128, DK * E], F16, kind="ExternalInput")
    onc = nc.dram_tensor("onc", [128, 1], F16, kind="ExternalInput")
    hT = nc.dram_tensor("hT", [128, DK * PC], F16, kind="ExternalOutput")
    d2X = nc.dram_tensor("d2X", [E, PC], F32, kind="ExternalOutput")
    rrO = nc.dram_tensor("rrO", [1, PC], F32, kind="ExternalOutput")
    wout = nc.dram_tensor("wout", [128, 512], F32, kind="ExternalOutput")

    NCH = PC // 512  # 512-column compute chunks

    with tile.TileContext(nc) as tc:
        with (
            tc.tile_pool(name="const", bufs=1) as cpool,
            tc.tile_pool(name="act", bufs=1) as apool,
            tc.tile_pool(name="ps", bufs=2, space="PSUM") as pspool,
            tc.tile_pool(name="ps_small", bufs=2, space="PSUM") as psmall,
        ):
            _warmup(nc, cpool, pspool, wout, "ps", NWARM1)

            # Inputs: xT per 512-column chunk (contiguous 4KB lines), weights
            # in one shot each, spread over the two HWDGE trigger queues.
            xT_sb = cpool.tile([128, DK, PC], F16, tag="xT")
            pw_sb = cpool.tile([128, DK, D], F16, tag="pw")
            nc.scalar.dma_start(pw_sb[:], pw[:])
            for n in range(NCH):
                nc.sync.dma_start(
                    xT_sb[:, :, n * 512 : (n + 1) * 512],
                    xT[:, n * DK * 512 : (n + 1) * DK * 512],
                )
            rw_sb = cpool.tile([128, DK, D], F16, tag="rw")
            nc.scalar.dma_start(rw_sb[:], rw[:])
            pb_sb = cpool.tile([128, DK], F32, tag="pb")
            nc.scalar.dma_start(pb_sb[:], pb[:])
            kT2_sb = cpool.tile([128, DK, E], F16, tag="kT2")
            nc.scalar.dma_start(kT2_sb[:], kT2[:])
            onc_sb = cpool.tile([128, 1], F16, tag="onc")
            nc.scalar.dma_start(onc_sb[:], onc[:])

            hT_sb = apool.tile([128, DK, PC], F16, tag="hT")
            rT_sb = apool.tile([128, DK, PC], F16, tag="rT")
            r2_sb = apool.tile([128, DK, PC], F16, tag="r2")
            rr_sb = apool.tile([1, PC], F32, tag="rr")
            d2_sb = apool.tile([E, PC], F32, tag="d2")

            # h^T = pw^T-mm(x^T) + pb ; r^T = rw^T-mm(h^T)
            for w_sb, src, dst, bias in (
                (pw_sb, xT_sb, hT_sb, pb_sb),
                (rw_sb, hT_sb, rT_sb, None),
            ):
                for m in range(DK):
                    for n in range(NCH):
                        ps = pspool.tile([128, 512], F32, tag="ps")
                        for k in range(DK):
                            nc.tensor.matmul(
                                ps[:],
                                w_sb[:, k, m * 128 : (m + 1) * 128],
                                src[:, k, n * 512 : (n + 1) * 512],
                                start=(k == 0),
                                stop=(k == DK - 1),
                            )
                        if bias is not None:
                            nc.scalar.activation(
                                dst[:, m, n * 512 : (n + 1) * 512], ps[:],
                                AF.Identity, bias=bias[:, m : m + 1],
                            )
                            eng = nc.sync if (m * NCH + n) % 2 else nc.scalar
                            eng.dma_start(
                                hT[:, m * PC + n * 512 : m * PC + (n + 1) * 512],
                                dst[:, m, n * 512 : (n + 1) * 512],
                            )
                        else:
                            nc.scalar.activation(
                                dst[:, m, n * 512 : (n + 1) * 512], ps[:], AF.Copy
                            )

            # rr = sum_d r^2 (ones-matmul over partition tiles)
            for kt in range(DK):
                nc.vector.tensor_mul(
                    r2_sb[:, kt, :], rT_sb[:, kt, :], rT_sb[:, kt, :]
                )
            for n in range(NCH):
                ps1 = psmall.tile([1, 512], F32, tag="ps1")
                for k in range(DK):
                    nc.tensor.matmul(
                        ps1[:], onc_sb[:], r2_sb[:, k, n * 512 : (n + 1) * 512],
                        start=(k == 0), stop=(k == DK - 1),
                    )
                nc.scalar.activation(rr_sb[:, n * 512 : (n + 1) * 512], ps1[:], AF.Copy)
            nc.scalar.dma_start(rrO[:], rr_sb[:])

            # d2X = (-2 keys) . r   (|k|^2 and rr are added on the host)
            for n in range(NCH):
                psA = psmall.tile([E, 512], F32, tag="psA")
                for k in range(DK):
                    nc.tensor.matmul(
                        psA[:], kT2_sb[:, k, :], rT_sb[:, k, n * 512 : (n + 1) * 512],
                        start=(k == 0), stop=(k == DK - 1),
                    )
                nc.scalar.activation(d2_sb[:, n * 512 : (n + 1) * 512], psA[:], AF.Copy)
            nc.sync.dma_start(d2X[:], d2_sb[:])
    nc.compile()
    return nc


def _phase2_nc(lens: tuple) -> bass.Bass:
    S = len(lens)
    offs = [0]
    for L in lens:
        offs.append(offs[-1] + L)
    Ctot = offs[-1]

    nc = bacc.Bacc("TRN2", target_bir_lowering=False, num_devices=NC)
    # Partition-major packed layouts (16KB DMA lines for weights).
    hseg = nc.dram_tensor("hseg", [128, DK * Ctot], F16, kind="ExternalInput")
    w1s = nc.dram_tensor("w1s", [S, 128, DK * H], F16, kind="ExternalInput")
    w2s = nc.dram_tensor("w2s", [S, 128, HK * D], F16, kind="ExternalInput")
    b1s = nc.dram_tensor("b1s", [128, S * HK], F32, kind="ExternalInput")
    b2s = nc.dram_tensor("b2s", [128, S * DK], F32, kind="ExternalInput")
    oseg = nc.dram_tensor("oseg", [128, DK * Ctot], F32, kind="ExternalOutput")
    wout = nc.dram_tensor("wout", [128, 512], F32, kind="ExternalOutput")

    with tile.TileContext(nc) as tc:
        with (
            tc.tile_pool(name="const", bufs=1) as cpool,
            tc.tile_pool(name="w1p", bufs=3) as w1p,
            tc.tile_pool(name="w2p", bufs=3) as w2p,
            tc.tile_pool(name="hp", bufs=3) as hp,
            tc.tile_pool(name="hidp", bufs=3) as hidp,
            tc.tile_pool(name="op", bufs=4) as op,
            tc.tile_pool(name="hid_ps", bufs=2, space="PSUM") as hidps,
            tc.tile_pool(name="out_ps", bufs=1, space="PSUM") as outps,
        ):
            _warmup(nc, cpool, hidps, wout, "hps", NWARM2)

            b1_sb = cpool.tile([128, S * HK], F32, tag="b1")
            nc.scalar.dma_start(b1_sb[:], b1s[:])
            b2_sb = cpool.tile([128, S * DK], F32, tag="b2")
            nc.scalar.dma_start(b2_sb[:], b2s[:])

            for s in range(S):
                Lc = lens[s]
                off = offs[s]
                ht = hp.tile([128, DK, 512], F16, tag="h")
                nc.sync.dma_start(
                    ht[:, :, :Lc], hseg[:, off * DK : off * DK + DK * Lc]
                )
                w1t = w1p.tile([128, DK, H], F16, tag="w1")
                nc.sync.dma_start(w1t[:, 0:2, :], w1s[s, :, : 2 * H])
                nc.scalar.dma_start(w1t[:, 2:4, :], w1s[s, :, 2 * H :])
                w2t = w2p.tile([128, HK, D], F16, tag="w2")
                nc.sync.dma_start(w2t[:, 0:8, :], w2s[s, :, : 8 * D])
                nc.scalar.dma_start(w2t[:, 8:16, :], w2s[s, :, 8 * D :])
                opsum = outps.tile([128, DK, 512], F32, tag="opsum", name=f"opsum_{s}")
                for m in range(HK):
                    hps = hidps.tile([128, 512], F32, tag="hps")
                    for k in range(DK):
                        nc.tensor.matmul(
                            hps[:, :Lc],
                            w1t[:, k, m * 128 : (m + 1) * 128],
                            ht[:, k, :Lc],
                            start=(k == 0),
                            stop=(k == DK - 1),
                        )
                    hidt = hidp.tile([128, 512], F16, tag="hid")
                    nc.scalar.activation(
                        hidt[:, :Lc], hps[:, :Lc], AF.Gelu,
                        bias=b1_sb[:, s * HK + m : s * HK + m + 1],
                    )
                    for mo in range(DK):
                        nc.tensor.matmul(
                            opsum[:, mo, :Lc],
                            w2t[:, m, mo * 128 : (mo + 1) * 128],
                            hidt[:, :Lc],
                            start=(m == 0),
                            stop=(m == HK - 1),
                        )
                # Drain PSUM with ACT+DVE in parallel; spread the output
                # writes over SWDGE + both HWDGE queues.
                for mo in range(DK):
                    ot = op.tile([128, 512], F32, tag="o")
                    bcol = b2_sb[:, s * DK + mo : s * DK + mo + 1]
                    if mo % 2:
                        nc.vector.tensor_scalar(
                            ot[:, :Lc], opsum[:, mo, :Lc], bcol, None, ALU.add
                        )
                    else:
                        nc.scalar.activation(
                            ot[:, :Lc], opsum[:, mo, :Lc], AF.Identity, bias=bcol
                        )
                    eng = (nc.gpsimd, nc.gpsimd, nc.sync, nc.scalar)[mo]
                    eng.dma_start(
                        oseg[:, off * DK + mo * Lc : off * DK + (mo + 1) * Lc],
                        ot[:, :Lc],
                    )
    nc.compile()
    return nc


def _run(nc, in_maps, label):
    trace = os.environ.get("KTRACE") == "1"
    res = run_bass_kernel_spmd(
        nc, in_maps, core_ids=list(range(NC)), trace=trace
    )
    if trace:
        last_stats[label] = {
            "exec_time_ns": res.exec_time_ns,
            "mean_exec_time_ns": res.mean_exec_time_ns,
            "trace": res.instructions_and_trace[1]
            if res.instructions_and_trace
            else None,
        }
    return res.results


def _round16(x: int) -> int:
    return (x + 15) & ~15


def _pack_pmajor(a: np.ndarray, kt: int) -> np.ndarray:
    """[kt*128, F] row-major -> [128, kt*F] partition-major (f16)."""
    f = a.shape[1]
    return np.ascontiguousarray(
        a.reshape(kt, 128, f).transpose(1, 0, 2).reshape(128, kt * f)
    )


def _plan_slots(topi, gates):
    """Cut each expert's selected columns into pieces and pick compile-time
    slot lengths minimizing the per-core column total (SPMD-uniform)."""
    per_e = []  # (token_ids, gate_vals)
    for e in range(E):
        sel_tok, sel_k = np.nonzero(topi == e)
        per_e.append((sel_tok, gates[sel_tok, sel_k]))

    full_pieces = []  # (e, start, 512)
    rem_pieces = []   # (e, start, len)
    for e in range(E):
        n = per_e[e][0].size
        f, r = divmod(n, 512)
        for i in range(f):
            full_pieces.append((e, i * 512, 512))
        if r:
            rem_pieces.append((e, f * 512, r))
    rem_pieces.sort(key=lambda p: -p[2])

    Fn = len(full_pieces)
    best = None
    for p in range(len(rem_pieces) + 1):
        n512 = Fn + p
        S512 = max(1, math.ceil(n512 / NC))
        rest = rem_pieces[p:]
        tlens = [
            max(MIN_SLOT, _round16(rest[i * NC][2]))
            for i in range(math.ceil(len(rest) / NC))
        ]
        cost = 512 * S512 + sum(tlens)
        if best is None or cost < best[0]:
            best = (cost, p, S512, tlens)
    _, p, S512, tlens = best

    full_cells = full_pieces + rem_pieces[:p]
    full_cells += [None] * (S512 * NC - len(full_cells))
    rest = rem_pieces[p:]
    slots = []  # (length, [8 cells])
    for s in range(S512):
        slots.append((512, full_cells[s * NC : (s + 1) * NC]))
    for i, tl in enumerate(tlens):
        cells = rest[i * NC : (i + 1) * NC]
        cells += [None] * (NC - len(cells))
        slots.append((tl, cells))  # short slots last: smaller kernel tail
    return per_e, slots


def kernel(view0, view1, proj_w, proj_b, router_w, expert_keys, w1, b1, w2, b2):
    view0 = np.ascontiguousarray(view0, dtype=np.float32)
    view1 = np.ascontiguousarray(view1, dtype=np.float32)
    proj_w = np.asarray(proj_w, dtype=np.float32)
    proj_b = np.asarray(proj_b, dtype=np.float32)
    router_w = np.asarray(router_w, dtype=np.float32)
    keys = np.asarray(expert_keys, dtype=np.float32)
    w1 = np.asarray(w1, dtype=np.float32)
    b1 = np.asarray(b1, dtype=np.float32)
    w2 = np.asarray(w2, dtype=np.float32)
    b2 = np.asarray(b2, dtype=np.float32)

    # ---- Phase 1: h, cross-term d2X and rr on device ----
    xT_full = np.concatenate(
        [view0.reshape(N, D).T, view1.reshape(N, D).T], axis=1
    ).astype(np.float16)  # [D, NT], column t = view*N + (b*T + tt)

    kT2 = _pack_pmajor(np.ascontiguousarray(-2.0 * keys.T).astype(np.float16), DK)
    kk = (keys * keys).sum(axis=1, dtype=np.float32)  # [E]
    onc = np.ones((128, 1), np.float16)

    in_maps1 = []
    for c in range(NC):
        v = (c * PC) // N  # cores 0-3 -> view 0, 4-7 -> view 1
        xc = xT_full[:, c * PC : (c + 1) * PC]  # [D, PC]
        # chunk-contiguous packing: [128, (n512, k, c)]
        xr = np.ascontiguousarray(
            xc.reshape(DK, 128, PC // 512, 512)
            .transpose(1, 2, 0, 3)
            .reshape(128, DK * PC)
        )
        in_maps1.append(
            {
                "xT": xr,
                "pw": _pack_pmajor(proj_w[v].astype(np.float16), DK),
                "pb": np.ascontiguousarray(proj_b[v].reshape(DK, 128).T),
                "rw": _pack_pmajor(router_w[v].astype(np.float16), DK),
                "kT2": kT2,
                "onc": onc,
            }
        )
    res1 = _run(_phase1_nc(), in_maps1, "phase1")

    # hT output layout [128, (m, col)] -> [D, PC] per core
    hT_d = np.concatenate(
        [
            r["hT"].reshape(128, DK, PC).transpose(1, 0, 2).reshape(D, PC)
            for r in res1
        ],
        axis=1,
    )  # [D, NT] f16
    d2 = np.concatenate([r["d2X"] for r in res1], axis=1).T      # [NT, E] f32
    rr = np.concatenate([r["rrO"] for r in res1], axis=1).T      # [NT, 1] f32
    d2 += rr
    d2 += kk[None, :]

    # ---- Host repair: recompute borderline tokens exactly in fp32 ----
    logits0 = -np.sqrt(np.maximum(d2, 0.0), dtype=np.float32)
    part = np.partition(logits0, E - K - 1, axis=1)
    gap45 = part[:, E - K] - part[:, E - K - 1]  # 4th minus 5th logit
    risk = np.nonzero(gap45 < REPAIR_MARGIN)[0]
    last_stats["n_repaired"] = int(risk.size)
    if risk.size:
        x_all = np.concatenate(
            [view0.reshape(N, D), view1.reshape(N, D)], axis=0
        )
        vsel = (risk >= N).astype(np.int64)
        for v in (0, 1):
            rt = risk[vsel == v]
            if rt.size == 0:
                continue
            hx = x_all[rt] @ proj_w[v] + proj_b[v]
            rx = hx @ router_w[v]
            d2[rt] = (
                (rx * rx).sum(axis=1, keepdims=True)
                - 2.0 * (rx @ keys.T)
                + kk
            )

    # ---- Host routing: logits, top-4, softmax gates (fp32) ----
    logits = -np.sqrt(np.maximum(d2, 0.0), dtype=np.float32)
    topi = np.argsort(-logits, axis=1, kind="stable")[:, :K]   # [NT, K]
    topv = np.take_along_axis(logits, topi, axis=1)
    ex = np.exp(topv - topv[:, :1], dtype=np.float32)
    gates = ex / ex.sum(axis=1, keepdims=True, dtype=np.float32)

    # ---- Slot plan ----
    per_e, slots = _plan_slots(topi, gates)
    lens = tuple(sl[0] for sl in slots)
    S = len(lens)
    offs = np.concatenate([[0], np.cumsum(lens)]).astype(np.int64)
    Ctot = int(offs[-1])

    # ---- Phase 2 inputs (partition-major packed) ----
    w1r = np.stack([_pack_pmajor(w1[e].astype(np.float16), DK) for e in range(E)])
    w2r = np.stack([_pack_pmajor(w2[e].astype(np.float16), HK) for e in range(E)])
    hT_p = np.ascontiguousarray(
        hT_d.reshape(DK, 128, NT).transpose(1, 0, 2)
    )  # [128, DK, NT]
    in_maps2 = []
    core_cells = []  # per core: list over slots of (e, toks, gvals) or None
    for c in range(NC):
        hseg = np.zeros((128, DK * Ctot), np.float16)
        w1c = np.zeros((S, 128, DK * H), np.float16)
        w2c = np.zeros((S, 128, HK * D), np.float16)
        b1c = np.zeros((128, S * HK), np.float32)
        b2c = np.zeros((128, S * DK), np.float32)
        cells = []
        for s, (Lc, cell8) in enumerate(slots):
            cell = cell8[c]
            if cell is None:
                cells.append(None)
                continue
            e, start, n = cell
            toks = per_e[e][0][start : start + n]
            gv = per_e[e][1][start : start + n]
            cells.append((e, toks, gv))
            blk = hT_p[:, :, toks]  # [128, DK, n]
            o0 = int(offs[s]) * DK
            hs = hseg[:, o0 : o0 + DK * Lc].reshape(128, DK, Lc)  # strided view
            hs[:, :, :n] = blk
            w1c[s] = w1r[e]
            w2c[s] = w2r[e]
            b1c[:, s * HK : (s + 1) * HK] = b1[e].reshape(HK, 128).T
            b2c[:, s * DK : (s + 1) * DK] = b2[e].reshape(DK, 128).T
        core_cells.append(cells)
        in_maps2.append(
            {"hseg": hseg, "w1s": w1c, "w2s": w2c, "b1s": b1c, "b2s": b2c}
        )
    last_stats["S"] = S
    last_stats["n_slots_real"] = sum(
        1 for cells in core_cells for cl in cells if cl is not None
    )
    last_stats["cols_per_core"] = Ctot
    res2 = _run(_phase2_nc(lens), in_maps2, "phase2")

    # ---- Combine (gates applied here) ----
    fusedT = np.zeros((D, NT), np.float32)
    for c in range(NC):
        o = res2[c]["oseg"]  # [128, DK*Ctot]
        for s in range(S):
            cell = core_cells[c][s]
            if cell is None:
                continue
            e, toks, gv = cell
            n = toks.size
            Lc = lens[s]
            o0 = int(offs[s]) * DK
            blk = o[:, o0 : o0 + DK * Lc].reshape(128, DK, Lc)[:, :, :n]
            fusedT[:, toks] += (
                blk.transpose(1, 0, 2).reshape(D, n) * gv[None, :]
            )
    fused = (fusedT[:, :N] + fusedT[:, N:]).T  # [N, D]
    return np.ascontiguousarray(fused.reshape(B, T, D), dtype=np.float32)
